# revision 1
# baseline (speedup 1.0000x reference)
"""Causal MHSA (RoPE) on 8 Trainium2 NeuronCores.

Sharding: core c = 2*b + g handles batch b (of 4) and head-group g (8 of 16
heads).  Each core projects Q/K/V for its heads, applies RoPE, runs causal
attention, then the two cores of a batch AllGather their (unnormalized)
context halves + per-head softmax denominators and each computes a disjoint
512-column slice of the output projection.

Device layouts (partition dim first):
  x^T   [128d, 8dsub, s]     streamed per 512-wide s-tile (PE transposes)
  Q^T/K^T [128e, 4et, 2048s]  e = head*64 + (even dk | odd dk)  (host-permuted
                              W columns so RoPE pairs are partition blocks)
  V_ext [128s, 16ks, 8h, 65]  per-head V plus a ones column (softmax denom)
  S^T   [128k, 2x512q] PSUM pairs -> one exp on ACT -> masked diag blocks (DVE)
  ctx^T [65, 512] accumulated in PSUM over k-subtiles (row 64 = denominator)
  ctx_own [8 heads x 65 rows, S] in DRAM: 64 unnormalized ctx rows + 1/den row
  out^T [128c, 512s] accumulated over e-subtiles of the gathered ctx

Softmax skips max-subtraction: scores = (x Wq)(x Wk)^T/8 with |S| < ~3 for
this problem's scale (Wq,Wk ~ 0.02 N(0,1)), so exp is safe in fp32.

All matmul operands are bf16 (1 cyc/row on the PE and roughly half the
switching power of float32r's dual LOW/HIGH row groups -- the activity-based
DVFS throttle was clamping the PE to 50% util for 87% of the attention
phase in the fp32r version).  PSUM accumulation stays fp32.
"""

import sys
from contextlib import ExitStack

for _p in ("/opt/trn_rl_repo",):
    if _p not in sys.path:
        sys.path.append(_p)

import ml_dtypes
import numpy as np

import concourse.bass as bass
import concourse.tile as tile
from concourse import bacc, mybir
from concourse.bass_utils import run_bass_kernel_spmd
from concourse.masks import make_identity

P = 128
S = 2048
D = 1024
NH = 16
DK = 64
HB = DK + 1    # per-head ctx block rows (64 ctx + 1 recip-den)
HPC = 8        # heads per core
EH = 512       # per-core head-dim total (8 heads * 64)
CTXR = HPC * HB  # 520 rows in the shipped ctx block
NCORES = 8
ST = 4         # s tiles of 512
DSUB = D // P  # 8
F32 = mybir.dt.float32
F32R = mybir.dt.float32r
BF16 = mybir.dt.bfloat16


def build_nc():
    nc = bacc.Bacc("TRN2", target_bir_lowering=False, debug=False,
                   num_devices=NCORES)

    x_sh = nc.declare_dram_parameter("x_sh", [S, D], F32, isOutput=False)
    wq = nc.declare_dram_parameter("wq", [D, EH], BF16, isOutput=False)
    wk = nc.declare_dram_parameter("wk", [D, EH], BF16, isOutput=False)
    wv = nc.declare_dram_parameter("wv", [D, EH], BF16, isOutput=False)
    wo = nc.declare_dram_parameter("wo", [D, EH], BF16, isOutput=False)
    cos_t = nc.declare_dram_parameter("cos_t", [P, S], BF16, isOutput=False)
    ssin_t = nc.declare_dram_parameter("ssin_t", [P, S], BF16, isOutput=False)
    out_t = nc.declare_dram_parameter("out_t", [EH, S], F32, isOutput=True)

    ctx_own = nc.dram_tensor("ctx_own", [EH, S], BF16)
    ctx_pieces = [
        nc.dram_tensor(f"ctx_g{j}", [2 * P, S], BF16) for j in range(4)
    ]

    with tile.TileContext(nc) as tc:
        _body(tc, x_sh, wq, wk, wv, wo, cos_t, ssin_t, out_t, ctx_own,
              ctx_pieces)
    nc.compile()  # Bacc: register allocation, DCE, nop/EVSEM fusion
    return nc


def _body(tc, x_sh, wq, wk, wv, wo, cos_t, ssin_t, out_t, ctx_own,
          ctx_pieces):
    nc = tc.nc

    ctx = ExitStack()
    with ctx:
        persist = ctx.enter_context(tc.tile_pool(name="persist", bufs=1))
        qkt_pool = ctx.enter_context(tc.tile_pool(name="qkt", bufs=1))
        wchp = ctx.enter_context(tc.tile_pool(name="wchp", bufs=3))
        expp = ctx.enter_context(tc.tile_pool(name="expp", bufs=3))
        xtsp = ctx.enter_context(tc.tile_pool(name="xtsp", bufs=1))

        # ---- persistent big tensors (declared first so DMAs can start) ----
        qkT = qkt_pool.tile([P, 8, S], BF16, name="qkT", tag="big64")
        v_ext = persist.tile([P, 16, HPC, HB], BF16, name="v_ext")
        nc.vector.memset(v_ext[:, :, :, :], 1.0)

        # ---- constants ----
        ident = persist.tile([P, P], BF16, name="ident")
        make_identity(nc, ident)
        # mask_m[p, u] = 1.0 iff u - p - 384 >= 0 (slices give the four
        # diagonal-block causal masks for S^T tiles)
        mask_f = persist.tile([P, 896], F32, name="mask_f")
        nc.gpsimd.memset(mask_f, 1.0)
        nc.gpsimd.affine_select(
            out=mask_f, in_=mask_f, compare_op=mybir.AluOpType.is_ge,
            fill=0.0, base=-384, pattern=[[1, 896]], channel_multiplier=-1,
        )
        # tril-complement [128,128] mask for ki-aligned diagonal blocks
        trim = persist.tile([P, P], BF16, name="trim")
        nc.vector.tensor_copy(out=trim, in_=mask_f[:, 384:512])
        # ones row for the K=1 denominator-broadcast matmul
        ones_t = persist.tile([HB, DK], F32, name="ones_t")
        nc.vector.memset(ones_t, 1.0)
        # partition-swap permutation (p <-> p^32) for RoPE, as free-dim
        # shifted copies of identity blocks (no cross-partition ops needed)
        swp_t = persist.tile([P, P], BF16, name="swp_t")
        nc.gpsimd.memset(swp_t, 0.0)
        for (r0, c0) in ((0, 32), (32, 0), (64, 96), (96, 64)):
            nc.gpsimd.tensor_copy(
                out=swp_t[r0:r0 + 32, c0:c0 + 32],
                in_=ident[r0:r0 + 32, r0:r0 + 32])

        # ================= phase 1: x^T, QKV =================
        with tc.tile_pool(name="ph1psum", bufs=4, space="PSUM") as ph1ps, \
             tc.tile_pool(name="tpsum", bufs=2, space="PSUM") as tpps, \
             tc.tile_pool(name="xstagep", bufs=1) as xstagep, \
             tc.tile_pool(name="xbfp", bufs=1) as xbfp, \
             tc.tile_pool(name="ropep", bufs=1) as ropep:
            first = True
            for st in range(ST):
                sl = slice(st * 512, (st + 1) * 512)
                xts = xtsp.tile([P, DSUB, 512], BF16, name="xts", tag="xts")
                stages = []
                for s128 in range(4):
                    r0 = st * 512 + s128 * P
                    x_stage = xstagep.tile([P, D], F32, name="x_stage",
                                           tag=f"xstage{s128}")
                    nc.sync.dma_start(out=x_stage, in_=x_sh[r0:r0 + P, :])
                    x_bf = xbfp.tile([P, D], BF16, name="x_bf",
                                     tag=f"xbf{s128}")
                    # fp32 -> bf16 cast on the (otherwise idle) ACT engine
                    nc.scalar.copy(out=x_bf, in_=x_stage)
                    stages.append(x_bf)
                if first:
                    # RoPE tables arrive behind the first x tile on purpose
                    cos_sb = persist.tile([P, S], BF16, name="cos_sb")
                    nc.sync.dma_start(out=cos_sb, in_=cos_t[:, :])
                    ssin_sb = persist.tile([P, S], BF16, name="ssin_sb")
                    nc.sync.dma_start(out=ssin_sb, in_=ssin_t[:, :])
                    first = False
                for dsub in range(DSUB):
                    tp4 = tpps.tile([P, 512], BF16, name="tp4", tag="tp")
                    for s128 in range(4):
                        nc.tensor.transpose(
                            tp4[:, s128 * P:(s128 + 1) * P],
                            stages[s128][:, dsub * P:(dsub + 1) * P], ident)
                    nc.vector.tensor_copy(out=xts[:, dsub, :], in_=tp4)

                # Q^T and K^T for this s-tile, then V
                for w_dram, qk_off in ((wq, 0), (wk, 4)):
                    pss = []
                    for dsub in range(DSUB):
                        wch = wchp.tile([P, EH], BF16, name="wch", tag="wch")
                        nc.sync.dma_start(
                            out=wch, in_=w_dram[dsub * P:(dsub + 1) * P, :])
                        for et in range(4):
                            if dsub == 0:
                                pss.append(ph1ps.tile(
                                    [P, 512], F32, name="psqk", tag="ph1"))
                            nc.tensor.matmul(
                                pss[et], lhsT=wch[:, et * P:(et + 1) * P],
                                rhs=xts[:, dsub, :],
                                start=(dsub == 0), stop=(dsub == DSUB - 1))
                    for et in range(4):
                        nc.vector.tensor_copy(
                            out=qkT[:, qk_off + et, sl], in_=pss[et])

                psv = []
                for dsub in range(DSUB):
                    wch = wchp.tile([P, EH], BF16, name="wch", tag="wch")
                    nc.sync.dma_start(
                        out=wch, in_=wv[dsub * P:(dsub + 1) * P, :])
                    for s128 in range(4):
                        if dsub == 0:
                            psv.append(ph1ps.tile(
                                [P, 512], F32, name="psv", tag="ph1"))
                        nc.tensor.matmul(
                            psv[s128],
                            lhsT=xts[:, dsub, s128 * P:(s128 + 1) * P],
                            rhs=wch,
                            start=(dsub == 0), stop=(dsub == DSUB - 1))
                for s128 in range(4):
                    nc.vector.tensor_copy(
                        out=v_ext[:, st * 4 + s128, :, 0:DK],
                        in_=psv[s128].rearrange("p (h d) -> p h d", h=HPC))

            # ---- RoPE, per 128-row block of Q^T/K^T ----
            # swapped rows via PE permutation matmul; combine on DVE
            for tb in (0, 4, 1, 5, 2, 6, 3, 7):  # Q/K per et
                dst = qkT[:, tb, :]
                tmp = ropep.tile([P, S], BF16, name="ropetmp", tag="rt")
                for c4 in range(4):
                    csl = slice(c4 * 512, (c4 + 1) * 512)
                    sw = tpps.tile([P, 512], F32, name="sw", tag="swp")
                    nc.tensor.matmul(sw, lhsT=swp_t, rhs=dst[:, csl],
                                     start=True, stop=True)
                    nc.vector.tensor_mul(tmp[:, csl], sw, ssin_sb[:, csl])
                nc.vector.tensor_mul(dst, dst, cos_sb)
                nc.vector.tensor_add(dst, dst, tmp)

        # ================= phase 2: attention =================
        with tc.tile_pool(name="spsum", bufs=2, space="PSUM") as sps, \
             tc.tile_pool(name="cpsum", bufs=3, space="PSUM") as cps, \
             tc.tile_pool(name="rbpsum", bufs=1, space="PSUM") as rbps, \
             tc.tile_pool(name="smallp", bufs=2) as smallp, \
             tc.tile_pool(name="normp", bufs=2) as normp:
            for et in range(4):
                for qt in range(ST):
                    qsl = slice(qt * 512, (qt + 1) * 512)
                    nk = 4 * (qt + 1)
                    pc = [cps.tile([HB, 512], F32, name="psctx", tag="ctx")
                          for _ in range(2)]
                    # off-diagonal k pairs (ki < 4*qt): fully unmasked
                    for kp in range(2 * qt):
                        for hh in range(2):  # head pair: parts 0-63 / 64-127
                            pb = hh * DK
                            ps2 = sps.tile([P, 1024], F32, name="pss", tag="ss")
                            for j in range(2):
                                ki = 2 * kp + j
                                ksl = slice(ki * P, (ki + 1) * P)
                                nc.tensor.matmul(
                                    ps2[:, j * 512:(j + 1) * 512],
                                    lhsT=qkT[pb:pb + DK, 4 + et, ksl],
                                    rhs=qkT[pb:pb + DK, et, qsl],
                                    start=True, stop=True)
                            ex2 = expp.tile([P, 1024], BF16, name="ex", tag="ex")
                            nc.scalar.activation(
                                out=ex2, in_=ps2,
                                func=mybir.ActivationFunctionType.Exp)
                            for j in range(2):
                                ki = 2 * kp + j
                                nc.tensor.matmul(
                                    pc[hh],
                                    lhsT=v_ext[:, ki, 2 * et + hh, :],
                                    rhs=ex2[:, j * 512:(j + 1) * 512],
                                    start=(ki == 0), stop=False)
                    # diagonal 512-block (ki = 4qt..4qt+3): scores start at
                    # q = 128*ki so only the leading [128,128] block of each
                    # k-subtile needs the tril mask; widths 512/384/256/128
                    for pair in range(2):
                        for hh in range(2):
                            pb = hh * DK
                            ps2 = sps.tile([P, 1024], F32, name="pss", tag="ss")
                            offs = []
                            off = 0
                            for j in range(2):
                                dki = 2 * pair + j
                                ki = 4 * qt + dki
                                w = 512 - 128 * dki
                                ksl = slice(ki * P, (ki + 1) * P)
                                nc.tensor.matmul(
                                    ps2[:, off:off + w],
                                    lhsT=qkT[pb:pb + DK, 4 + et, ksl],
                                    rhs=qkT[pb:pb + DK, et,
                                            qt * 512 + 128 * dki:
                                            (qt + 1) * 512],
                                    start=True, stop=True)
                                offs.append((off, w, dki, ki))
                                off += w
                            ex2 = expp.tile([P, 1024], BF16, name="ex", tag="ex")
                            nc.scalar.activation(
                                out=ex2[:, 0:off], in_=ps2[:, 0:off],
                                func=mybir.ActivationFunctionType.Exp)
                            for (o, w, dki, ki) in offs:
                                nc.vector.tensor_mul(
                                    ex2[:, o:o + P], ex2[:, o:o + P], trim)
                                nc.tensor.matmul(
                                    pc[hh][:, 128 * dki:512],
                                    lhsT=v_ext[:, ki, 2 * et + hh, :],
                                    rhs=ex2[:, o:o + w],
                                    start=(ki == 0), stop=(ki == nk - 1))
                    # normalize: 1/den broadcast via K=1 matmul, then scale
                    for hh in range(2):
                        h_loc = 2 * et + hh
                        rec = smallp.tile([HB, 512], F32, name="rec", tag="rec")
                        nc.vector.reciprocal(out=rec[DK:HB, :],
                                             in_=pc[hh][DK:HB, :])
                        rb = rbps.tile([DK, 512], F32, name="rb", tag="rb")
                        nc.tensor.matmul(
                            rb, lhsT=ones_t[DK:HB, :],
                            rhs=rec[DK:HB, :], start=True, stop=True)
                        cstx = normp.tile([DK, 512], BF16, name="cstx", tag="cstx")
                        nc.vector.tensor_copy(out=cstx, in_=rb)
                        nc.vector.tensor_mul(cstx, cstx, pc[hh][0:DK, :])
                        nc.sync.dma_start(
                            out=ctx_own[h_loc * DK:(h_loc + 1) * DK, qsl],
                            in_=cstx)
                # one AG per completed head-pair, overlapped with later ets
                nc.gpsimd.collective_compute(
                    "AllGather",
                    mybir.AluOpType.bypass,
                    replica_groups=[[0, 1], [2, 3], [4, 5], [6, 7]],
                    ins=[ctx_own[et * P:(et + 1) * P, :]],
                    outs=[ctx_pieces[et][:, :]],
                )

        # ================= phase 4: output projection =================
        with tc.tile_pool(name="opsum", bufs=8, space="PSUM") as ops, \
             tc.tile_pool(name="ctxchp", bufs=3) as ctxchp, \
             tc.tile_pool(name="outstp", bufs=4) as outstp:
            wo_sb = xtsp.tile([P, DSUB, EH], BF16, name="wo_sb", tag="xts")
            nc.sync.dma_start(out=wo_sb, in_=wo[:, :].rearrange(
                "(o p) n -> p o n", p=P))

            for st in range(ST):
                sl = slice(st * 512, (st + 1) * 512)
                po = []
                for ei, esub in enumerate((0, 4, 1, 5, 2, 6, 3, 7)):
                    piece = ctx_pieces[esub % 4]
                    r0 = (esub // 4) * P
                    ch = ctxchp.tile([P, 512], BF16, name="ctxch", tag="cch")
                    nc.sync.dma_start(
                        out=ch, in_=piece[r0:r0 + P, sl])
                    for ct in range(4):
                        if ei == 0:
                            po.append(ops.tile([P, 512], F32, name="pso", tag="po"))
                        nc.tensor.matmul(
                            po[ct], lhsT=wo_sb[:, esub, ct * P:(ct + 1) * P],
                            rhs=ch,
                            start=(ei == 0), stop=(ei == DSUB - 1))
                for ct in range(4):
                    ost = outstp.tile([P, 512], F32, name="ost", tag="ost")
                    nc.vector.tensor_copy(out=ost, in_=po[ct])
                    nc.sync.dma_start(
                        out=out_t[ct * P:(ct + 1) * P, sl], in_=ost)


_NC_CACHE = None


def _get_nc():
    global _NC_CACHE
    if _NC_CACHE is None:
        _NC_CACHE = build_nc()
    return _NC_CACHE


def _prep_in_maps(x, token_positions, Wq, Wk, Wv, Wo):
    x = np.asarray(x, np.float32)
    Wq = np.asarray(Wq, np.float32)
    Wk = np.asarray(Wk, np.float32)
    Wv = np.asarray(Wv, np.float32)
    Wo = np.asarray(Wo, np.float32)
    pos = np.asarray(token_positions).astype(np.float32)

    half = DK // 2
    inv_freq = (1.0 / (10000.0 ** (np.arange(half, dtype=np.float32) * 2.0 / DK))
                ).astype(np.float32)
    ang = pos[:, None] * inv_freq[None, :]          # [S, 32] fp32
    cosT = np.cos(ang).T.astype(np.float32)         # [32, S]
    sinT = np.sin(ang).T.astype(np.float32)
    cos128 = np.ascontiguousarray(np.tile(cosT, (4, 1))).astype(
        ml_dtypes.bfloat16)                                         # [128, S]
    ssin128 = np.ascontiguousarray(
        np.concatenate([-sinT, sinT, -sinT, sinT], axis=0)).astype(
        ml_dtypes.bfloat16)                                         # [128, S]

    # within-head column permutation: [even dk dims, odd dk dims]
    perm = np.concatenate([np.arange(0, DK, 2), np.arange(1, DK, 2)])
    in_maps = []
    for c in range(NCORES):
        b, g = c // 2, c % 2
        heads = np.arange(g * HPC, (g + 1) * HPC)
        qk_cols = np.concatenate([h * DK + perm for h in heads])
        vsl = slice(g * EH, (g + 1) * EH)
        bf = ml_dtypes.bfloat16
        in_maps.append({
            "x_sh": np.ascontiguousarray(x[b]),
            "wq": np.ascontiguousarray(
                Wq[:, qk_cols] * np.float32(0.125)).astype(bf),
            "wk": np.ascontiguousarray(Wk[:, qk_cols]).astype(bf),
            "wv": np.ascontiguousarray(Wv[:, vsl]).astype(bf),
            "wo": np.ascontiguousarray(Wo[:, vsl]).astype(bf),
            "cos_t": cos128,
            "ssin_t": ssin128,
        })
    return in_maps


def kernel(x, token_positions, Wq, Wk, Wv, Wo, _trace=False, _trace_kwargs=None):
    in_maps = _prep_in_maps(x, token_positions, Wq, Wk, Wv, Wo)
    nc = _get_nc()
    res = run_bass_kernel_spmd(
        nc, in_maps, core_ids=list(range(NCORES)),
        trace=_trace, **(_trace_kwargs or {}))
    B = np.asarray(x).shape[0]
    out = np.empty((B, S, D), np.float32)
    for c in range(NCORES):
        b, g = c // 2, c % 2
        out[b, :, g * EH:(g + 1) * EH] = res.results[c]["out_t"].T
    if _trace:
        return out, res
    return out

